# revision 35
# baseline (speedup 1.0000x reference)
"""DeepGAT (4-layer GAT + BN + residual + MLP head) on 8 Trainium2 cores.

Sharding: nodes are dst-partitioned across the 8 cores (1250 nodes/core).
Edges are routed on the host to the core owning their dst node and sorted by
dst. Weights are replicated. Per layer each core projects all N nodes in bf16
(head-minor column layout), writes per-node rows [xl' 1024 | alpha_src 8] to
its local DRAM table, then gathers the rows of its edges' src nodes with one
dma_gather per dst block. The per-edge one-hot matrices (edge->dst scatter)
and their transposes (dst->edge gather, used to broadcast alpha_dst to edges
on the PE) are precomputed on the host and streamed from DRAM. Segment
softmax is normalized AFTER aggregation (mathematically identical), so the
edge phase is a single pass. h is exchanged with a bf16 AllGather.
"""

import numpy as np
import ml_dtypes

import concourse.bass as bass
import concourse.bacc as bacc
import concourse.mybir as mybir
from concourse.tile import TileContext
from concourse.tile_rust import add_dep_helper

FP32 = mybir.dt.float32
BF16 = mybir.dt.bfloat16
I16 = mybir.dt.int16
AF = mybir.ActivationFunctionType
OP = mybir.AluOpType
AX = mybir.AxisListType
NPBF16 = ml_dtypes.bfloat16
NPFP8 = ml_dtypes.float8_e4m3fn
FP8 = mybir.dt.float8e4

# problem constants (hardcoded per harness contract)
ALPHA = 0.1
BN_EPS = 1e-5
NEG_SLOPE = 0.2
HID = 128  # partition width; fixed


def _set_dims(n=10000, e=160000, in_dim=512, heads=8, layers=4, cls=2, cores=8):
    """Set problem dims as module globals (parametrized for sim tests)."""
    g = globals()
    g["N"], g["E"], g["IN"], g["H"], g["L"], g["CLS"], g["M"] = (
        n, e, in_dim, heads, layers, cls, cores)
    g["NPC"] = n // cores
    g["NPC_PAD"] = -(-g["NPC"] // 128) * 128
    g["NBLK"] = g["NPC_PAD"] // 128
    g["N_PAD"] = -(-n // 128) * 128
    g["NNB"] = g["N_PAD"] // 128
    g["HC"] = heads * HID
    g["PRJ"] = g["HC"] + heads          # xl' 1024 | alpha_src 8
    g["ROW"] = -(-g["PRJ"] // 128) * 128  # gather elem (mult of 128 elems)


_set_dims()


def _hblk_pieces(nb):
    """Global node block nb -> [(core, local_lo, col_lo, width)] from agout,
    plus zero-fill width for pad nodes (>= N)."""
    g0, g1 = nb * 128, min(nb * 128 + 128, N)
    out = []
    k0, k1 = g0 // NPC, (g1 - 1) // NPC
    for k in range(k0, k1 + 1):
        lo, hi = max(g0, k * NPC), min(g1, (k + 1) * NPC)
        if hi > lo:
            out.append((k, lo - k * NPC, lo - g0, hi - lo))
    return out, g1 - g0


HALF_NODE = 5120  # proj blocks [0,40) cover global nodes [0, 5120)
SPLIT_BLKS = 0    # first SPLIT_BLKS dst blocks get src-half split gathers


class Cfg:
    """Static schedule computed from the actual edge data."""

    def __init__(self, chunks_per_block):
        self.chunks_per_block = list(chunks_per_block)
        self.CH = sum(self.chunks_per_block)
        self.CBMAX = max(self.chunks_per_block)
        self.TOTE = 128 * self.CH
        self.ca = [0] * SPLIT_BLKS  # whole-chunks guaranteed src<HALF_NODE


def _pack_idx16(idx, pad_to=None):
    """Pack int16 indices for dma_gather: idx i at [i%16, i//16], replicated
    to 128 partitions."""
    idx = np.asarray(idx, np.int64)
    n = len(idx)
    if pad_to is not None:
        assert pad_to >= n
        idx = np.concatenate([idx, np.zeros(pad_to - n, np.int64)])
        n = pad_to
    assert n % 16 == 0
    a = idx.astype(np.int16).reshape(n // 16, 16).T  # [16, n//16]
    return np.tile(a, (8, 1)).copy()  # [128, n//16]


def preprocess(x, edge_index, Wp, bp, Wl, att_src, att_dst, bl, gamma, beta,
               W1, b1, W2, b2):
    """Host-side: edge routing/sorting per core, one-hot tables, weight
    folding (head-minor column permutation)."""
    x = np.asarray(x, np.float32)
    src = np.concatenate([np.asarray(edge_index[0]), np.arange(N)]).astype(np.int64)
    dst = np.concatenate([np.asarray(edge_index[1]), np.arange(N)]).astype(np.int64)

    per_core = []
    for k in range(M):
        m = (dst // NPC) == k
        s_k, d_k = src[m], dst[m] - k * NPC
        order = np.argsort(d_k, kind="stable")
        per_core.append((s_k[order], d_k[order]))

    counts = np.zeros((M, NBLK), np.int64)
    for k in range(M):
        _, d_k = per_core[k]
        b = d_k // 128
        for bb in range(NBLK):
            counts[k, bb] = int((b == bb).sum())
    chunks_per_block = [max(1, int(np.ceil(counts[:, bb].max() / 128)))
                        for bb in range(NBLK)]
    cfg = Cfg(chunks_per_block)

    ca_min = [10 ** 9] * SPLIT_BLKS
    per_core_inputs = []
    for k in range(M):
        s_k, d_k = per_core[k]
        b_k = d_k // 128
        srcidx = np.zeros(cfg.TOTE, np.int64)
        oh_all = np.zeros((128, cfg.CH * 128), NPFP8)
        ohT_all = np.zeros((128, cfg.CH * 128), NPFP8)
        off = 0  # chunk offset
        for bb in range(NBLK):
            sel = b_k == bb
            cnt = int(sel.sum())
            cap = 128 * cfg.chunks_per_block[bb]
            assert cnt <= cap, (k, bb, cnt, cap)
            s_sel, d_sel = s_k[sel], d_k[sel]
            if bb < SPLIT_BLKS:
                # src-half groups so an early gather can cover group A
                order = np.argsort(s_sel >= HALF_NODE, kind="stable")
                s_sel, d_sel = s_sel[order], d_sel[order]
                ca_min[bb] = min(ca_min[bb],
                                 int((s_sel < HALF_NODE).sum()) // 128)
            slots = off * 128 + np.arange(cnt)
            srcidx[slots[0]:slots[0] + cnt] = s_sel
            dloc = d_sel - 128 * bb             # 0..127 within block
            ch = slots // 128                   # absolute chunk id
            ep = slots % 128                    # edge lane within chunk
            oh_all[ep, ch * 128 + dloc] = 1
            ohT_all[dloc, ch * 128 + ep] = 1
            off += cfg.chunks_per_block[bb]
        assert off == cfg.CH

        xT_own = np.zeros((IN, NPC_PAD), NPBF16)
        xT_own[:, :NPC] = x[k * NPC:(k + 1) * NPC].T

        per_core_inputs.append({
            "srcidx": _pack_idx16(srcidx),
            "oh_all": oh_all,
            "ohT_all": ohT_all,
            "xT_own": xT_own,
        })
    cfg.ca = [min(c, cfg.chunks_per_block[i]) for i, c in enumerate(ca_min)]

    # weight folding; head-minor permutation: col c*H+h <- h*HID+c
    Wl = np.asarray(Wl, np.float32)          # [L, HID, HC]
    a_s = np.asarray(att_src, np.float32)    # [L, H, HID]
    a_d = np.asarray(att_dst, np.float32)
    perm = (np.arange(HC).reshape(H, HID).T).reshape(-1)  # [c*H+h] -> h*HID+c
    Wcat = np.zeros((L, HID, PRJ), np.float32)
    Wadst = np.zeros((L, HID, H), np.float32)
    for i in range(L):
        Wcat[i, :, :HC] = Wl[i][:, perm]
        w3 = Wl[i].reshape(HID, H, HID)
        Wcat[i, :, HC:HC + H] = np.einsum("khc,hc->kh", w3, a_s[i])
        Wadst[i] = np.einsum("khc,hc->kh", w3, a_d[i])

    bn_inv = 1.0 / np.sqrt(1.0 + BN_EPS)
    gamma = np.asarray(gamma, np.float32)
    beta = np.asarray(beta, np.float32)
    bl = np.asarray(bl, np.float32)
    # h = elu((1-a)*(gamma*bn_inv*(mean+bl)+beta) + a*prev); fold 1/H into s.
    s_aff = ((1.0 - ALPHA) * gamma * bn_inv / H).T.copy()            # [HID, L]
    t_aff = ((1.0 - ALPHA) * (gamma * bn_inv * bl + beta)).T.copy()  # [HID, L]

    ident = np.eye(128, dtype=np.float32)

    shared = {
        "Wp": np.asarray(Wp, np.float32).astype(NPBF16),
        "bp": np.asarray(bp, np.float32)[:, None],
        "Wcat": Wcat.astype(NPBF16),
        "Wadst": np.ascontiguousarray(
            Wadst.transpose(1, 0, 2).reshape(HID, L * H)).astype(NPBF16),
        "s_aff": s_aff, "t_aff": t_aff,
        "W1": np.asarray(W1, np.float32).astype(NPBF16),
        "b1": np.asarray(b1, np.float32)[:, None],
        "W2": np.asarray(W2, np.float32).astype(NPBF16),
        "b2": np.asarray(b2, np.float32)[:, None],
        "ident": ident,
    }
    return cfg, shared, per_core_inputs


def _elu(nc, p, out_ap, z_ap, shape, tg):
    """out = elu(z) = relu(z) + exp(min(z,0)) - 1, z in SBUF f32.
    Processes in 512-col pieces so temp tiles stay small."""
    P, F = shape
    for j0 in range(0, F, 512):
        j1 = min(j0 + 512, F)
        w = j1 - j0
        mn = p.tile([P, 512], FP32, tag=f"elu_mn_{tg}")
        ex = p.tile([P, 512], FP32, tag=f"elu_ex_{tg}")
        rl = p.tile([P, 512], FP32, tag=f"elu_rl_{tg}")
        nc.vector.tensor_scalar_min(out=mn[:, :w], in0=z_ap[:, j0:j1],
                                    scalar1=0.0)
        nc.scalar.activation(out=ex[:, :w], in_=mn[:, :w], func=AF.Exp)
        nc.vector.tensor_scalar_max(out=rl[:, :w], in0=z_ap[:, j0:j1],
                                    scalar1=0.0)
        nc.vector.tensor_tensor(out=rl[:, :w], in0=rl[:, :w], in1=ex[:, :w],
                                op=OP.add)
        nc.vector.tensor_scalar_sub(out=out_ap[:, j0:j1], in0=rl[:, :w],
                                    scalar1=1.0)


def build(nc, cfg):
    """Emit the SPMD program (dims from module globals)."""
    n, n_pad, npc, npc_pad = N, N_PAD, NPC, NPC_PAD
    in_dim, layers, heads, cores = IN, L, H, M
    nblk, nnb, hc, prj = NBLK, NNB, HC, PRJ
    qd = HID // 2
    cbmax = cfg.CBMAX

    # ---------------- I/O ----------------
    srcidx = nc.dram_tensor("srcidx", [128, cfg.TOTE // 16], I16, kind="ExternalInput")
    oh_in = nc.dram_tensor("oh_all", [128, cfg.CH * 128], FP8, kind="ExternalInput")
    ohT_in = nc.dram_tensor("ohT_all", [128, cfg.CH * 128], FP8, kind="ExternalInput")
    xT_own_in = nc.dram_tensor("xT_own", [in_dim, npc_pad], BF16, kind="ExternalInput")
    Wp_in = nc.dram_tensor("Wp", [in_dim, HID], BF16, kind="ExternalInput")
    bp_in = nc.dram_tensor("bp", [HID, 1], FP32, kind="ExternalInput")
    Wcat_in = nc.dram_tensor("Wcat", [layers, HID, prj], BF16, kind="ExternalInput")
    Wadst_in = nc.dram_tensor("Wadst", [HID, layers * heads], BF16, kind="ExternalInput")
    s_aff_in = nc.dram_tensor("s_aff", [HID, layers], FP32, kind="ExternalInput")
    t_aff_in = nc.dram_tensor("t_aff", [HID, layers], FP32, kind="ExternalInput")
    W1_in = nc.dram_tensor("W1", [HID, qd], BF16, kind="ExternalInput")
    b1_in = nc.dram_tensor("b1", [qd, 1], FP32, kind="ExternalInput")
    W2_in = nc.dram_tensor("W2", [qd, CLS], BF16, kind="ExternalInput")
    b2_in = nc.dram_tensor("b2", [CLS, 1], FP32, kind="ExternalInput")
    ident_in = nc.dram_tensor("ident", [128, 128], FP32, kind="ExternalInput")
    out_dram = nc.dram_tensor("out", [CLS, npc_pad], FP32, kind="ExternalOutput")

    agout = nc.dram_tensor("h_agout", [cores * HID, npc_pad], BF16,
                           addr_space="Shared" if cores > 4 else "Local")

    with TileContext(nc) as tc:
        with (
            tc.tile_pool(name="const", bufs=1) as cpool,
            tc.tile_pool(name="hbuf", bufs=1) as hpool,
            tc.tile_pool(name="proj", bufs=2) as ppool,
            tc.tile_pool(name="gath", bufs=3) as gpool,
            tc.tile_pool(name="ohp", bufs=2) as ohpool,
            tc.tile_pool(name="edge", bufs=3) as epool,
            tc.tile_pool(name="blk", bufs=2) as bpool,
            tc.tile_pool(name="dram", bufs=1, space="DRAM") as dpool,
            tc.tile_pool(name="ps", bufs=2, space="PSUM") as psS,
        ):
            # dma_gather allocates a register per distinct count; cache them
            _regs = {}

            def nreg(v):
                if v not in _regs:
                    _regs[v] = nc.gpsimd.to_reg(v)
                return _regs[v]

            # ---------------- resident constants / state ----------------
            ident_f = cpool.tile([128, 128], FP32)
            nc.sync.dma_start(out=ident_f[:], in_=ident_in[:, :])
            srcidx_sb = cpool.tile([128, cfg.TOTE // 16], I16)
            nc.sync.dma_start(out=srcidx_sb[:], in_=srcidx[:, :])
            s_aff = cpool.tile([128, layers], FP32)
            nc.sync.dma_start(out=s_aff[:], in_=s_aff_in[:, :])
            t_aff = cpool.tile([128, layers], FP32)
            nc.sync.dma_start(out=t_aff[:], in_=t_aff_in[:, :])
            Wadst_sb = cpool.tile([128, layers * heads], BF16)
            nc.sync.dma_start(out=Wadst_sb[:], in_=Wadst_in[:, :])
            W1_sb = cpool.tile([128, qd], BF16)
            nc.sync.dma_start(out=W1_sb[:], in_=W1_in[:, :])
            b1_sb = cpool.tile([qd, 1], FP32)
            nc.sync.dma_start(out=b1_sb[:], in_=b1_in[:, :])
            W2_sb = cpool.tile([qd, CLS], BF16)
            nc.sync.dma_start(out=W2_sb[:], in_=W2_in[:, :])
            b2_sb = cpool.tile([CLS, 1], FP32)
            nc.sync.dma_start(out=b2_sb[:], in_=b2_in[:, :])
            bp_sb = cpool.tile([HID, 1], FP32)
            nc.sync.dma_start(out=bp_sb[:], in_=bp_in[:, :])

            h_own = [hpool.tile([128, npc_pad], BF16, tag=f"h_own{i}",
                                name=f"h_own{i}")
                     for i in range(2)]

            kchunks = in_dim // 128

            # ------- h0 = elu(x @ Wp + bp), own nodes only (scoped pool) ----
            with tc.tile_pool(name="x0", bufs=2) as x0pool:
                Wp_sb = cpool.tile([128, kchunks, HID], BF16)
                for kc in range(kchunks):
                    nc.sync.dma_start(out=Wp_sb[:, kc, :],
                                      in_=Wp_in[kc * 128:(kc + 1) * 128, :])
                z0 = bpool.tile([128, npc_pad], BF16, tag="z2a", bufs=1)
                for j0 in range(0, npc_pad, 512):
                    j1 = min(j0 + 512, npc_pad)
                    ps = psS.tile([128, 1024], FP32, tag="agg", name="h0_ps")
                    for kc in range(kchunks):
                        xt = x0pool.tile([128, 512], BF16, tag="xT",
                                         name="xt")
                        nc.sync.dma_start(
                            out=xt[:, :j1 - j0],
                            in_=xT_own_in[kc * 128:(kc + 1) * 128, j0:j1])
                        nc.tensor.matmul(out=ps[:, :j1 - j0],
                                         lhsT=Wp_sb[:, kc, :],
                                         rhs=xt[:, :j1 - j0],
                                         start=(kc == 0),
                                         stop=(kc == kchunks - 1))
                    nc.scalar.activation(out=z0[:, j0:j1], in_=ps[:, :j1 - j0],
                                         func=AF.Identity,
                                         bias=bp_sb[:, :1], scale=1.0)
                _elu(nc, bpool, h_own[0][:], z0[:], (128, npc_pad), "n")

            # ---------------- layers ----------------
            for li in range(layers):
                hprev = h_own[li % 2]
                hnew = h_own[(li + 1) % 2]

                # --- alpha_dst for own nodes (independent of AllGather) ---
                ad_own = bpool.tile([128, nblk * heads], BF16, tag="ad_own")
                for bb in range(nblk):
                    adp = psS.tile([128, 128], FP32, tag="mT", name="adp")
                    nc.tensor.matmul(
                        out=adp[:, :heads],
                        lhsT=hprev[:, bb * 128:(bb + 1) * 128],
                        rhs=Wadst_sb[:, li * heads:(li + 1) * heads],
                        start=True, stop=True)
                    nc.vector.tensor_copy(
                        out=ad_own[:, bb * heads:(bb + 1) * heads],
                        in_=adp[:, :heads])

                # --- allgather h (own cols -> full agout) ---
                bounce = dpool.tile([HID, npc_pad], BF16, tag="bounce")
                nc.sync.dma_start(out=bounce[:], in_=hprev[:])
                cc = nc.gpsimd.collective_compute(
                    "AllGather", OP.bypass,
                    replica_groups=[list(range(cores))],
                    ins=[bounce[:]], outs=[agout[:, :]],
                )


                # --- projection: all nodes, xl' | alpha_src (head-minor) ---
                Wc = ppool.tile([128, prj], BF16, tag="Wc")
                nc.sync.dma_start(out=Wc[:], in_=Wcat_in[li, :, :])
                xlrow_t = dpool.tile([n_pad, ROW], BF16, tag="xlrow")
                tbl_writes = []
                for nb in range(nnb):
                    if nb % 2 == 0:
                        hblk2 = ppool.tile([128, 256], BF16, tag="hblk",
                                           bufs=3)
                        g0 = nb * 128
                        g1 = min(g0 + 256, n)
                        if g1 - g0 < 256:
                            nc.vector.memset(hblk2[:, g1 - g0:], 0.0)
                        k0, k1 = g0 // npc, (g1 - 1) // npc
                        for k in range(k0, k1 + 1):
                            lo = max(g0, k * npc)
                            hi = min(g1, (k + 1) * npc)
                            if hi <= lo:
                                continue
                            d = nc.sync.dma_start(
                                out=hblk2[:, lo - g0:hi - g0],
                                in_=agout[k * HID:(k + 1) * HID,
                                          lo - k * npc:hi - k * npc])
                            add_dep_helper(d.ins, cc.ins, True, "cc")
                    hblk = hblk2[:, (nb % 2) * 128:(nb % 2 + 1) * 128]
                    if nb % 2 == 0:
                        xlwr2 = ppool.tile([128, 2, prj], BF16, tag="xlwr")
                    xlwr = xlwr2[:, nb % 2, :]
                    ppA = psS.tile([128, hc], FP32, tag="agg", name="ppA")
                    ppB = psS.tile([128, hc], FP32, tag="agg", name="ppB")
                    pa = psS.tile([128, 128], FP32, tag="mT", name="pa")
                    nc.tensor.matmul(out=ppA[:, :512], lhsT=hblk,
                                     rhs=Wc[:, 0:512],
                                     start=True, stop=True,
                                     skip_group_check=True)
                    nc.tensor.matmul(out=ppB[:, :512], lhsT=hblk,
                                     rhs=Wc[:, 512:1024],
                                     start=True, stop=True,
                                     skip_group_check=True)
                    nc.tensor.matmul(out=pa[:, :heads],
                                     lhsT=hblk,
                                     rhs=Wc[:, hc:prj],
                                     start=True, stop=True)
                    nc.scalar.activation(out=xlwr[:, :512],
                                         in_=ppA[:, :512], func=AF.Copy)
                    nc.vector.tensor_copy(out=xlwr[:, 512:hc],
                                          in_=ppB[:, :512])
                    nc.scalar.activation(out=xlwr[:, hc:prj],
                                         in_=pa[:, :heads], func=AF.Copy)
                    if nb % 2 == 1 or nb == nnb - 1:
                        nb0 = nb - nb % 2
                        nbk = nb % 2 + 1
                        w_ = nc.sync.dma_start(
                            out=xlrow_t[nb0 * 128:(nb0 + nbk) * 128, :prj]
                            .rearrange("(b p) c -> p b c", b=nbk),
                            in_=xlwr2[:, :nbk, :])
                        tbl_writes.append(w_)

                # --- edge phase, per dst block; epilogue spread over the
                # next two block iterations so every op's deps are ready
                # long before its engine reaches it (in-order queues) ---
                DN = cbmax * heads  # den columns start in ad_den

                hmall = bpool.tile([128, npc_pad], FP32, tag="hmall",
                                   bufs=1)

                def stageA(st):  # DVE: rec / hm8 / head-reduce -> hmall
                    bb = st["bb"]
                    rec = bpool.tile([128, heads], FP32, tag="rec")
                    # clamp: pad dst lanes have denom 0
                    nc.vector.tensor_scalar_max(
                        out=rec[:], in0=st["ad_den"][:, DN:DN + heads],
                        scalar1=1e-20)
                    nc.vector.reciprocal(out=rec[:], in_=rec[:])
                    hm8 = bpool.tile([128, hc], BF16, tag="hm8", bufs=1)
                    rec_b = (rec[:].rearrange("p (a b) -> p a b", a=1)
                             .to_broadcast([128, HID, heads]))
                    nc.vector.tensor_tensor(
                        out=hm8[:].rearrange("p (a b) -> p a b", a=HID),
                        in0=st["agg"][:].rearrange("p (a b) -> p a b", a=HID),
                        in1=rec_b, op=OP.mult)
                    nc.vector.tensor_reduce(
                        out=hmall[:, bb * 128:(bb + 1) * 128],
                        in_=hm8[:].rearrange("p (a b) -> p a b", a=HID),
                        axis=AX.X, op=OP.add)

                p1 = None  # state of block bb-1
                off = 0
                for bb in range(nblk):
                    cb = cfg.chunks_per_block[bb]
                    ohc = ohpool.tile([128, cbmax * 128], FP8, tag="oh")
                    nc.sync.dma_start(
                        out=ohc[:, :cb * 128],
                        in_=oh_in[:, off * 128:(off + cb) * 128])
                    ohTc = ohpool.tile([128, cbmax * 128], FP8, tag="ohT")
                    nc.sync.dma_start(
                        out=ohTc[:, :cb * 128],
                        in_=ohT_in[:, off * 128:(off + cb) * 128])

                    gt = gpool.tile([128, cbmax, ROW], BF16, tag="gt")
                    ca = cfg.ca[bb] if bb < SPLIT_BLKS else 0
                    if 0 < ca < cb:
                        ga = nc.gpsimd.dma_gather(
                            out_ap=gt[:, :ca, :], in_ap=xlrow_t[:],
                            idxs_ap=srcidx_sb[:, off * 8:(off + ca) * 8],
                            num_idxs=128 * ca, num_idxs_reg=nreg(128 * ca),
                            elem_size=ROW, single_packet=128 * ca <= 1024)
                        for w_ in tbl_writes[:40]:
                            add_dep_helper(ga.ins, w_.ins, True, "tblA->g")
                        g_ = nc.gpsimd.dma_gather(
                            out_ap=gt[:, ca:cb, :], in_ap=xlrow_t[:],
                            idxs_ap=srcidx_sb[:, (off + ca) * 8:(off + cb) * 8],
                            num_idxs=128 * (cb - ca),
                            num_idxs_reg=nreg(128 * (cb - ca)),
                            elem_size=ROW,
                            single_packet=128 * (cb - ca) <= 1024)
                        for w_ in tbl_writes:
                            add_dep_helper(g_.ins, w_.ins, True, "tbl->g")
                    else:
                        g_ = nc.gpsimd.dma_gather(
                            out_ap=gt[:, :cb, :], in_ap=xlrow_t[:],
                            idxs_ap=srcidx_sb[:, off * 8:(off + cb) * 8],
                            num_idxs=128 * cb, num_idxs_reg=nreg(128 * cb),
                            elem_size=ROW, single_packet=128 * cb <= 1024)
                        for w_ in tbl_writes:
                            add_dep_helper(g_.ins, w_.ins, True, "tbl->gather")

                    if p1 is not None:
                        stageA(p1)

                    # per-edge alpha_dst via transposed one-hot, batched PSUM
                    ad_den = psS.tile([128, cbmax * heads + heads], FP32,
                                      tag="ad_den")
                    for j in range(cb):
                        nc.tensor.matmul(
                            out=ad_den[:, j * heads:(j + 1) * heads],
                            lhsT=ohTc[:, j * 128:(j + 1) * 128],
                            rhs=ad_own[:, bb * heads:(bb + 1) * heads],
                            start=True, stop=True, skip_group_check=True)
                    sv_all = epool.tile([128, cbmax * heads], BF16, tag="sv")
                    nc.vector.tensor_tensor(
                        out=sv_all[:, :cb * heads].rearrange(
                            "p (a b) -> p a b", a=cb),
                        in0=gt[:, :cb, hc:hc + heads],
                        in1=ad_den[:, :cb * heads].rearrange(
                            "p (a b) -> p a b", a=cb),
                        op=OP.add)
                    # pe = exp(lrelu(sv)); lrelu = max(x, 0.2x) on DVE
                    lr_all = epool.tile([128, cbmax * heads], BF16, tag="lr")
                    nc.vector.tensor_scalar_mul(out=lr_all[:, :cb * heads],
                                                in0=sv_all[:, :cb * heads],
                                                scalar1=NEG_SLOPE)
                    nc.vector.tensor_tensor(out=lr_all[:, :cb * heads],
                                            in0=sv_all[:, :cb * heads],
                                            in1=lr_all[:, :cb * heads],
                                            op=OP.max)
                    pe_all = epool.tile([128, cbmax * heads], BF16, tag="pe")
                    nc.scalar.activation(out=pe_all[:, :cb * heads],
                                         in_=lr_all[:, :cb * heads],
                                         func=AF.Exp)

                    agg = psS.tile([128, hc], FP32, tag="agg")
                    for j in range(cb):
                        first, last = j == 0, j == cb - 1
                        # msg[e, (c,h)] = xl'[e, (c,h)] * pe[e, h]
                        msg = epool.tile([128, hc], BF16, tag="msg", bufs=4)
                        pe_b = (pe_all[:, j * heads:(j + 1) * heads]
                                .rearrange("p (a b) -> p a b", a=1)
                                .to_broadcast([128, HID, heads]))
                        nc.vector.tensor_tensor(
                            out=msg[:].rearrange("p (a b) -> p a b", a=HID),
                            in0=gt[:, j, :hc].rearrange(
                                "p (a b) -> p a b", a=HID),
                            in1=pe_b, op=OP.mult)
                        nc.tensor.matmul(out=ad_den[:, DN:DN + heads],
                                         lhsT=ohc[:, j * 128:(j + 1) * 128],
                                         rhs=pe_all[:, j * heads:(j + 1) * heads],
                                         start=first, stop=last,
                                         skip_group_check=True)
                        for j0 in range(0, hc, 512):
                            j1 = min(j0 + 512, hc)
                            nc.tensor.matmul(out=agg[:, j0:j1],
                                             lhsT=ohc[:, j * 128:(j + 1) * 128],
                                             rhs=msg[:, j0:j1],
                                             start=first, stop=last,
                                             skip_group_check=True)
                    off += cb
                    p1 = {"bb": bb, "agg": agg, "ad_den": ad_den}
                stageA(p1)

                # --- batched layer epilogue over all own nodes ---
                mTall = bpool.tile([128, npc_pad], BF16, tag="mTall",
                                   bufs=1)
                for bb in range(nblk):
                    mT_ps = psS.tile([128, 128], FP32, tag="mT",
                                     name="mT_ps")
                    nc.tensor.transpose(
                        out=mT_ps[:], in_=hmall[:, bb * 128:(bb + 1) * 128],
                        identity=ident_f[:])
                    if bb % 2 == 0:
                        nc.scalar.activation(
                            out=mTall[:, bb * 128:(bb + 1) * 128],
                            in_=mT_ps[:], func=AF.Copy)
                    else:
                        nc.vector.tensor_copy(
                            out=mTall[:, bb * 128:(bb + 1) * 128],
                            in_=mT_ps[:])
                z1a = bpool.tile([128, npc_pad], FP32, tag="z1a", bufs=1)
                nc.scalar.activation(out=z1a[:], in_=mTall[:],
                                     func=AF.Identity,
                                     bias=t_aff[:, li:li + 1],
                                     scale=s_aff[:, li:li + 1])
                z2a = bpool.tile([128, npc_pad], FP32, tag="z2a", bufs=1)
                nc.vector.tensor_scalar_mul(out=z2a[:], in0=hprev[:],
                                            scalar1=ALPHA)
                nc.vector.tensor_tensor(out=z1a[:], in0=z1a[:], in1=z2a[:],
                                        op=OP.add)
                _elu(nc, bpool, hnew[:], z1a[:], (128, npc_pad), "n")

            # ---------------- classifier ----------------
            hfin = h_own[layers % 2]
            zc = bpool.tile([qd, npc_pad], BF16, tag="z2a", bufs=1)
            for j0 in range(0, npc_pad, 512):
                j1 = min(j0 + 512, npc_pad)
                hid_ps = psS.tile([qd, 1024], FP32, tag="agg",
                                  name="hid_ps")
                nc.tensor.matmul(out=hid_ps[:, :j1 - j0], lhsT=W1_sb[:],
                                 rhs=hfin[:, j0:j1], start=True, stop=True)
                nc.scalar.activation(out=zc[:, j0:j1], in_=hid_ps[:, :j1 - j0],
                                     func=AF.Identity,
                                     bias=b1_sb[:, :1], scale=1.0)
            hidsb = bpool.tile([qd, npc_pad], BF16, tag="mTall", bufs=1)
            _elu(nc, bpool, hidsb[:], zc[:], (qd, npc_pad), "n")
            osb = bpool.tile([CLS, npc_pad], FP32, tag="z1a", bufs=1)
            for j0 in range(0, npc_pad, 512):
                j1 = min(j0 + 512, npc_pad)
                out_ps = psS.tile([CLS, 1024], FP32, tag="agg",
                                  name="out_ps")
                nc.tensor.matmul(out=out_ps[:, :j1 - j0], lhsT=W2_sb[:],
                                 rhs=hidsb[:, j0:j1], start=True, stop=True)
                nc.scalar.activation(out=osb[:, j0:j1], in_=out_ps[:, :j1 - j0],
                                     func=AF.Identity,
                                     bias=b2_sb[:, :1], scale=1.0)
            nc.sync.dma_start(out=out_dram[:, :], in_=osb[:])

    return nc


_LAST_EXEC_NS = None


def _run(inputs, trace=False):
    global _LAST_EXEC_NS
    from concourse.bass_utils import run_bass_kernel_spmd

    cfg, shared, per_core = preprocess(**inputs)
    nc = bacc.Bacc("TRN2", target_bir_lowering=False, debug=False,
                   num_devices=M)
    build(nc, cfg)
    nc.compile()

    in_maps = []
    for k in range(M):
        m = dict(shared)
        m.update(per_core[k])
        in_maps.append({k2: np.ascontiguousarray(v) for k2, v in m.items()})

    res = run_bass_kernel_spmd(nc, in_maps, list(range(M)), trace=trace)
    _LAST_EXEC_NS = res.exec_time_ns

    out = np.zeros((N, CLS), np.float32)
    for k in range(M):
        o = res.results[k]["out"]  # [CLS, NPC_PAD]
        out[k * NPC:(k + 1) * NPC] = o[:CLS, :NPC].T
    return out


def kernel(**inputs):
    return _run(inputs, trace=False)


# revision 36
# speedup vs baseline: 1.0924x; 1.0924x over previous
"""DeepGAT (4-layer GAT + BN + residual + MLP head) on 8 Trainium2 cores.

Sharding: nodes are dst-partitioned across the 8 cores (1250 nodes/core).
Edges are routed on the host to the core owning their dst node and sorted by
dst. Weights are replicated. Per layer each core projects all N nodes in bf16
(head-minor column layout), writes per-node rows [xl' 1024 | alpha_src 8] to
its local DRAM table, then gathers the rows of its edges' src nodes with one
dma_gather per dst block. The per-edge one-hot matrices (edge->dst scatter)
and their transposes (dst->edge gather, used to broadcast alpha_dst to edges
on the PE) are precomputed on the host and streamed from DRAM. Segment
softmax is normalized AFTER aggregation (mathematically identical), so the
edge phase is a single pass. h is exchanged with a bf16 AllGather.
"""

import numpy as np
import ml_dtypes

import concourse.bass as bass
import concourse.bacc as bacc
import concourse.mybir as mybir
from concourse.tile import TileContext
from concourse.tile_rust import add_dep_helper

FP32 = mybir.dt.float32
BF16 = mybir.dt.bfloat16
I16 = mybir.dt.int16
AF = mybir.ActivationFunctionType
OP = mybir.AluOpType
AX = mybir.AxisListType
NPBF16 = ml_dtypes.bfloat16
NPFP8 = ml_dtypes.float8_e4m3fn
FP8 = mybir.dt.float8e4

# problem constants (hardcoded per harness contract)
ALPHA = 0.1
BN_EPS = 1e-5
NEG_SLOPE = 0.2
HID = 128  # partition width; fixed


def _set_dims(n=10000, e=160000, in_dim=512, heads=8, layers=4, cls=2, cores=8):
    """Set problem dims as module globals (parametrized for sim tests)."""
    g = globals()
    g["N"], g["E"], g["IN"], g["H"], g["L"], g["CLS"], g["M"] = (
        n, e, in_dim, heads, layers, cls, cores)
    g["NPC"] = n // cores
    g["NPC_PAD"] = -(-g["NPC"] // 128) * 128
    g["NBLK"] = g["NPC_PAD"] // 128
    g["N_PAD"] = -(-n // 128) * 128
    g["NNB"] = g["N_PAD"] // 128
    g["HC"] = heads * HID
    g["PRJ"] = g["HC"] + heads          # xl' 1024 | alpha_src 8
    g["ROW"] = -(-g["PRJ"] // 128) * 128  # gather elem (mult of 128 elems)


_set_dims()


def _hblk_pieces(nb):
    """Global node block nb -> [(core, local_lo, col_lo, width)] from agout,
    plus zero-fill width for pad nodes (>= N)."""
    g0, g1 = nb * 128, min(nb * 128 + 128, N)
    out = []
    k0, k1 = g0 // NPC, (g1 - 1) // NPC
    for k in range(k0, k1 + 1):
        lo, hi = max(g0, k * NPC), min(g1, (k + 1) * NPC)
        if hi > lo:
            out.append((k, lo - k * NPC, lo - g0, hi - lo))
    return out, g1 - g0


HALF_NODE = 5120  # proj blocks [0,40) cover global nodes [0, 5120)
SPLIT_BLKS = 0    # first SPLIT_BLKS dst blocks get src-half split gathers


class Cfg:
    """Static schedule computed from the actual edge data."""

    def __init__(self, chunks_per_block):
        self.chunks_per_block = list(chunks_per_block)
        self.CH = sum(self.chunks_per_block)
        self.CBMAX = max(self.chunks_per_block)
        self.TOTE = 128 * self.CH
        self.ca = [0] * SPLIT_BLKS  # whole-chunks guaranteed src<HALF_NODE


def _pack_idx16(idx, pad_to=None):
    """Pack int16 indices for dma_gather: idx i at [i%16, i//16], replicated
    to 128 partitions."""
    idx = np.asarray(idx, np.int64)
    n = len(idx)
    if pad_to is not None:
        assert pad_to >= n
        idx = np.concatenate([idx, np.zeros(pad_to - n, np.int64)])
        n = pad_to
    assert n % 16 == 0
    a = idx.astype(np.int16).reshape(n // 16, 16).T  # [16, n//16]
    return np.tile(a, (8, 1)).copy()  # [128, n//16]


def preprocess(x, edge_index, Wp, bp, Wl, att_src, att_dst, bl, gamma, beta,
               W1, b1, W2, b2):
    """Host-side: edge routing/sorting per core, one-hot tables, weight
    folding (head-minor column permutation)."""
    x = np.asarray(x, np.float32)
    src = np.concatenate([np.asarray(edge_index[0]), np.arange(N)]).astype(np.int64)
    dst = np.concatenate([np.asarray(edge_index[1]), np.arange(N)]).astype(np.int64)

    per_core = []
    for k in range(M):
        m = (dst // NPC) == k
        s_k, d_k = src[m], dst[m] - k * NPC
        order = np.argsort(d_k, kind="stable")
        per_core.append((s_k[order], d_k[order]))

    counts = np.zeros((M, NBLK), np.int64)
    for k in range(M):
        _, d_k = per_core[k]
        b = d_k // 128
        for bb in range(NBLK):
            counts[k, bb] = int((b == bb).sum())
    chunks_per_block = [max(1, int(np.ceil(counts[:, bb].max() / 128)))
                        for bb in range(NBLK)]
    cfg = Cfg(chunks_per_block)

    ca_min = [10 ** 9] * SPLIT_BLKS
    per_core_inputs = []
    for k in range(M):
        s_k, d_k = per_core[k]
        b_k = d_k // 128
        srcidx = np.zeros(cfg.TOTE, np.int64)
        oh_both = np.zeros((128, 2 * cfg.CH * 128), NPFP8)
        off = 0  # chunk offset
        for bb in range(NBLK):
            sel = b_k == bb
            cnt = int(sel.sum())
            cap = 128 * cfg.chunks_per_block[bb]
            assert cnt <= cap, (k, bb, cnt, cap)
            s_sel, d_sel = s_k[sel], d_k[sel]
            if bb < SPLIT_BLKS:
                # src-half groups so an early gather can cover group A
                order = np.argsort(s_sel >= HALF_NODE, kind="stable")
                s_sel, d_sel = s_sel[order], d_sel[order]
                ca_min[bb] = min(ca_min[bb],
                                 int((s_sel < HALF_NODE).sum()) // 128)
            slots = off * 128 + np.arange(cnt)
            srcidx[slots[0]:slots[0] + cnt] = s_sel
            dloc = d_sel - 128 * bb             # 0..127 within block
            cb_ = cfg.chunks_per_block[bb]
            rel = slots // 128 - off            # chunk id within block
            ep = slots % 128                    # edge lane within chunk
            base = 2 * off * 128                # block region: [oh | ohT]
            oh_both[ep, base + rel * 128 + dloc] = 1
            oh_both[dloc, base + (cb_ + rel) * 128 + ep] = 1
            off += cb_
        assert off == cfg.CH

        xT_own = np.zeros((IN, NPC_PAD), NPBF16)
        xT_own[:, :NPC] = x[k * NPC:(k + 1) * NPC].T

        per_core_inputs.append({
            "srcidx": _pack_idx16(srcidx),
            "oh_both": oh_both,
            "xT_own": xT_own,
        })
    cfg.ca = [min(c, cfg.chunks_per_block[i]) for i, c in enumerate(ca_min)]

    # weight folding; head-minor permutation: col c*H+h <- h*HID+c
    Wl = np.asarray(Wl, np.float32)          # [L, HID, HC]
    a_s = np.asarray(att_src, np.float32)    # [L, H, HID]
    a_d = np.asarray(att_dst, np.float32)
    perm = (np.arange(HC).reshape(H, HID).T).reshape(-1)  # [c*H+h] -> h*HID+c
    Wcat = np.zeros((L, HID, PRJ), np.float32)
    Wadst = np.zeros((L, HID, H), np.float32)
    for i in range(L):
        Wcat[i, :, :HC] = Wl[i][:, perm]
        w3 = Wl[i].reshape(HID, H, HID)
        Wcat[i, :, HC:HC + H] = np.einsum("khc,hc->kh", w3, a_s[i])
        Wadst[i] = np.einsum("khc,hc->kh", w3, a_d[i])

    bn_inv = 1.0 / np.sqrt(1.0 + BN_EPS)
    gamma = np.asarray(gamma, np.float32)
    beta = np.asarray(beta, np.float32)
    bl = np.asarray(bl, np.float32)
    # h = elu((1-a)*(gamma*bn_inv*(mean+bl)+beta) + a*prev); fold 1/H into s.
    s_aff = ((1.0 - ALPHA) * gamma * bn_inv / H).T.copy()            # [HID, L]
    t_aff = ((1.0 - ALPHA) * (gamma * bn_inv * bl + beta)).T.copy()  # [HID, L]

    ident = np.eye(128, dtype=np.float32)

    shared = {
        "Wp": np.asarray(Wp, np.float32).astype(NPBF16),
        "bp": np.asarray(bp, np.float32)[:, None],
        "Wcat": Wcat.astype(NPBF16),
        "Wadst": np.ascontiguousarray(
            Wadst.transpose(1, 0, 2).reshape(HID, L * H)).astype(NPBF16),
        "s_aff": s_aff, "t_aff": t_aff,
        "W1": np.asarray(W1, np.float32).astype(NPBF16),
        "b1": np.asarray(b1, np.float32)[:, None],
        "W2": np.asarray(W2, np.float32).astype(NPBF16),
        "b2": np.asarray(b2, np.float32)[:, None],
        "ident": ident,
    }
    return cfg, shared, per_core_inputs


def _elu(nc, p, out_ap, z_ap, shape, tg):
    """out = elu(z) = relu(z) + exp(min(z,0)) - 1, z in SBUF f32.
    Processes in 512-col pieces so temp tiles stay small."""
    P, F = shape
    for j0 in range(0, F, 512):
        j1 = min(j0 + 512, F)
        w = j1 - j0
        mn = p.tile([P, 512], FP32, tag=f"elu_mn_{tg}")
        ex = p.tile([P, 512], FP32, tag=f"elu_ex_{tg}")
        rl = p.tile([P, 512], FP32, tag=f"elu_rl_{tg}")
        nc.vector.tensor_scalar_min(out=mn[:, :w], in0=z_ap[:, j0:j1],
                                    scalar1=0.0)
        nc.scalar.activation(out=ex[:, :w], in_=mn[:, :w], func=AF.Exp)
        nc.vector.tensor_scalar_max(out=rl[:, :w], in0=z_ap[:, j0:j1],
                                    scalar1=0.0)
        nc.vector.tensor_tensor(out=rl[:, :w], in0=rl[:, :w], in1=ex[:, :w],
                                op=OP.add)
        nc.vector.tensor_scalar_sub(out=out_ap[:, j0:j1], in0=rl[:, :w],
                                    scalar1=1.0)


def build(nc, cfg):
    """Emit the SPMD program (dims from module globals)."""
    n, n_pad, npc, npc_pad = N, N_PAD, NPC, NPC_PAD
    in_dim, layers, heads, cores = IN, L, H, M
    nblk, nnb, hc, prj = NBLK, NNB, HC, PRJ
    qd = HID // 2
    cbmax = cfg.CBMAX

    # ---------------- I/O ----------------
    srcidx = nc.dram_tensor("srcidx", [128, cfg.TOTE // 16], I16, kind="ExternalInput")
    ohb_in = nc.dram_tensor("oh_both", [128, 2 * cfg.CH * 128], FP8,
                            kind="ExternalInput")
    xT_own_in = nc.dram_tensor("xT_own", [in_dim, npc_pad], BF16, kind="ExternalInput")
    Wp_in = nc.dram_tensor("Wp", [in_dim, HID], BF16, kind="ExternalInput")
    bp_in = nc.dram_tensor("bp", [HID, 1], FP32, kind="ExternalInput")
    Wcat_in = nc.dram_tensor("Wcat", [layers, HID, prj], BF16, kind="ExternalInput")
    Wadst_in = nc.dram_tensor("Wadst", [HID, layers * heads], BF16, kind="ExternalInput")
    s_aff_in = nc.dram_tensor("s_aff", [HID, layers], FP32, kind="ExternalInput")
    t_aff_in = nc.dram_tensor("t_aff", [HID, layers], FP32, kind="ExternalInput")
    W1_in = nc.dram_tensor("W1", [HID, qd], BF16, kind="ExternalInput")
    b1_in = nc.dram_tensor("b1", [qd, 1], FP32, kind="ExternalInput")
    W2_in = nc.dram_tensor("W2", [qd, CLS], BF16, kind="ExternalInput")
    b2_in = nc.dram_tensor("b2", [CLS, 1], FP32, kind="ExternalInput")
    ident_in = nc.dram_tensor("ident", [128, 128], FP32, kind="ExternalInput")
    out_dram = nc.dram_tensor("out", [CLS, npc_pad], FP32, kind="ExternalOutput")

    agout = nc.dram_tensor("h_agout", [cores * HID, npc_pad], BF16,
                           addr_space="Shared" if cores > 4 else "Local")

    with TileContext(nc) as tc:
        with (
            tc.tile_pool(name="const", bufs=1) as cpool,
            tc.tile_pool(name="hbuf", bufs=1) as hpool,
            tc.tile_pool(name="proj", bufs=2) as ppool,
            tc.tile_pool(name="gath", bufs=3) as gpool,
            tc.tile_pool(name="ohp", bufs=2) as ohpool,
            tc.tile_pool(name="edge", bufs=3) as epool,
            tc.tile_pool(name="blk", bufs=2) as bpool,
            tc.tile_pool(name="dram", bufs=1, space="DRAM") as dpool,
            tc.tile_pool(name="ps", bufs=2, space="PSUM") as psS,
        ):
            # dma_gather allocates a register per distinct count; cache them
            _regs = {}

            def nreg(v):
                if v not in _regs:
                    _regs[v] = nc.gpsimd.to_reg(v)
                return _regs[v]

            # ---------------- resident constants / state ----------------
            ident_f = cpool.tile([128, 128], FP32)
            nc.sync.dma_start(out=ident_f[:], in_=ident_in[:, :])
            srcidx_sb = cpool.tile([128, cfg.TOTE // 16], I16)
            nc.sync.dma_start(out=srcidx_sb[:], in_=srcidx[:, :])
            s_aff = cpool.tile([128, layers], FP32)
            nc.sync.dma_start(out=s_aff[:], in_=s_aff_in[:, :])
            t_aff = cpool.tile([128, layers], FP32)
            nc.sync.dma_start(out=t_aff[:], in_=t_aff_in[:, :])
            Wadst_sb = cpool.tile([128, layers * heads], BF16)
            nc.sync.dma_start(out=Wadst_sb[:], in_=Wadst_in[:, :])
            W1_sb = cpool.tile([128, qd], BF16)
            nc.sync.dma_start(out=W1_sb[:], in_=W1_in[:, :])
            b1_sb = cpool.tile([qd, 1], FP32)
            nc.sync.dma_start(out=b1_sb[:], in_=b1_in[:, :])
            W2_sb = cpool.tile([qd, CLS], BF16)
            nc.sync.dma_start(out=W2_sb[:], in_=W2_in[:, :])
            b2_sb = cpool.tile([CLS, 1], FP32)
            nc.sync.dma_start(out=b2_sb[:], in_=b2_in[:, :])
            bp_sb = cpool.tile([HID, 1], FP32)
            nc.sync.dma_start(out=bp_sb[:], in_=bp_in[:, :])

            h_own = [hpool.tile([128, npc_pad], BF16, tag=f"h_own{i}",
                                name=f"h_own{i}")
                     for i in range(2)]

            kchunks = in_dim // 128

            # ------- h0 = elu(x @ Wp + bp), own nodes only (scoped pool) ----
            with tc.tile_pool(name="x0", bufs=2) as x0pool:
                Wp_sb = cpool.tile([128, kchunks, HID], BF16)
                for kc in range(kchunks):
                    nc.sync.dma_start(out=Wp_sb[:, kc, :],
                                      in_=Wp_in[kc * 128:(kc + 1) * 128, :])
                z0 = bpool.tile([128, npc_pad], BF16, tag="z2a", bufs=1)
                for j0 in range(0, npc_pad, 512):
                    j1 = min(j0 + 512, npc_pad)
                    ps = psS.tile([128, 1024], FP32, tag="agg", name="h0_ps")
                    for kc in range(kchunks):
                        xt = x0pool.tile([128, 512], BF16, tag="xT",
                                         name="xt")
                        nc.sync.dma_start(
                            out=xt[:, :j1 - j0],
                            in_=xT_own_in[kc * 128:(kc + 1) * 128, j0:j1])
                        nc.tensor.matmul(out=ps[:, :j1 - j0],
                                         lhsT=Wp_sb[:, kc, :],
                                         rhs=xt[:, :j1 - j0],
                                         start=(kc == 0),
                                         stop=(kc == kchunks - 1))
                    nc.scalar.activation(out=z0[:, j0:j1], in_=ps[:, :j1 - j0],
                                         func=AF.Identity,
                                         bias=bp_sb[:, :1], scale=1.0)
                _elu(nc, bpool, h_own[0][:], z0[:], (128, npc_pad), "n")

            # ---------------- layers ----------------
            for li in range(layers):
                hprev = h_own[li % 2]
                hnew = h_own[(li + 1) % 2]

                # --- alpha_dst for own nodes (independent of AllGather) ---
                ad_own = bpool.tile([128, nblk * heads], BF16, tag="ad_own")
                for bb in range(nblk):
                    adp = psS.tile([128, 128], FP32, tag="mT", name="adp")
                    nc.tensor.matmul(
                        out=adp[:, :heads],
                        lhsT=hprev[:, bb * 128:(bb + 1) * 128],
                        rhs=Wadst_sb[:, li * heads:(li + 1) * heads],
                        start=True, stop=True)
                    nc.vector.tensor_copy(
                        out=ad_own[:, bb * heads:(bb + 1) * heads],
                        in_=adp[:, :heads])

                # --- allgather h (own cols -> full agout) ---
                bounce = dpool.tile([HID, npc_pad], BF16, tag="bounce")
                nc.sync.dma_start(out=bounce[:], in_=hprev[:])
                cc = nc.gpsimd.collective_compute(
                    "AllGather", OP.bypass,
                    replica_groups=[list(range(cores))],
                    ins=[bounce[:]], outs=[agout[:, :]],
                )


                # --- projection: all nodes, xl' | alpha_src (head-minor) ---
                Wc = ppool.tile([128, prj], BF16, tag="Wc")
                nc.sync.dma_start(out=Wc[:], in_=Wcat_in[li, :, :])
                xlrow_t = dpool.tile([n_pad, ROW], BF16, tag="xlrow")
                tbl_writes = []
                for nb in range(nnb):
                    if nb % 2 == 0:
                        hblk2 = ppool.tile([128, 256], BF16, tag="hblk",
                                           bufs=3)
                        g0 = nb * 128
                        g1 = min(g0 + 256, n)
                        if g1 - g0 < 256:
                            nc.vector.memset(hblk2[:, g1 - g0:], 0.0)
                        k0, k1 = g0 // npc, (g1 - 1) // npc
                        for k in range(k0, k1 + 1):
                            lo = max(g0, k * npc)
                            hi = min(g1, (k + 1) * npc)
                            if hi <= lo:
                                continue
                            d = nc.sync.dma_start(
                                out=hblk2[:, lo - g0:hi - g0],
                                in_=agout[k * HID:(k + 1) * HID,
                                          lo - k * npc:hi - k * npc])
                            add_dep_helper(d.ins, cc.ins, True, "cc")
                    hblk = hblk2[:, (nb % 2) * 128:(nb % 2 + 1) * 128]
                    if nb % 2 == 0:
                        xlwr2 = ppool.tile([128, 2, prj], BF16, tag="xlwr")
                    xlwr = xlwr2[:, nb % 2, :]
                    ppA = psS.tile([128, hc], FP32, tag="agg", name="ppA")
                    ppB = psS.tile([128, hc], FP32, tag="agg", name="ppB")
                    pa = psS.tile([128, 128], FP32, tag="mT", name="pa")
                    nc.tensor.matmul(out=ppA[:, :512], lhsT=hblk,
                                     rhs=Wc[:, 0:512],
                                     start=True, stop=True,
                                     skip_group_check=True)
                    nc.tensor.matmul(out=ppB[:, :512], lhsT=hblk,
                                     rhs=Wc[:, 512:1024],
                                     start=True, stop=True,
                                     skip_group_check=True)
                    nc.tensor.matmul(out=pa[:, :heads],
                                     lhsT=hblk,
                                     rhs=Wc[:, hc:prj],
                                     start=True, stop=True)
                    nc.scalar.activation(out=xlwr[:, :512],
                                         in_=ppA[:, :512], func=AF.Copy)
                    nc.vector.tensor_copy(out=xlwr[:, 512:hc],
                                          in_=ppB[:, :512])
                    nc.scalar.activation(out=xlwr[:, hc:prj],
                                         in_=pa[:, :heads], func=AF.Copy)
                    if nb % 2 == 1 or nb == nnb - 1:
                        nb0 = nb - nb % 2
                        nbk = nb % 2 + 1
                        w_ = nc.sync.dma_start(
                            out=xlrow_t[nb0 * 128:(nb0 + nbk) * 128, :prj]
                            .rearrange("(b p) c -> p b c", b=nbk),
                            in_=xlwr2[:, :nbk, :])
                        tbl_writes.append(w_)

                # --- edge phase, per dst block; epilogue spread over the
                # next two block iterations so every op's deps are ready
                # long before its engine reaches it (in-order queues) ---
                DN = cbmax * heads  # den columns start in ad_den

                hmall = bpool.tile([128, npc_pad], FP32, tag="hmall",
                                   bufs=1)

                def stageA(st):  # DVE: rec / hm8 / head-reduce -> hmall
                    bb = st["bb"]
                    rec = bpool.tile([128, heads], FP32, tag="rec")
                    # clamp: pad dst lanes have denom 0
                    nc.vector.tensor_scalar_max(
                        out=rec[:], in0=st["ad_den"][:, DN:DN + heads],
                        scalar1=1e-20)
                    nc.vector.reciprocal(out=rec[:], in_=rec[:])
                    hm8 = bpool.tile([128, hc], BF16, tag="hm8", bufs=1)
                    rec_b = (rec[:].rearrange("p (a b) -> p a b", a=1)
                             .to_broadcast([128, HID, heads]))
                    nc.vector.tensor_tensor(
                        out=hm8[:].rearrange("p (a b) -> p a b", a=HID),
                        in0=st["agg"][:].rearrange("p (a b) -> p a b", a=HID),
                        in1=rec_b, op=OP.mult)
                    nc.vector.tensor_reduce(
                        out=hmall[:, bb * 128:(bb + 1) * 128],
                        in_=hm8[:].rearrange("p (a b) -> p a b", a=HID),
                        axis=AX.X, op=OP.add)

                p1 = None  # state of block bb-1
                off = 0
                for bb in range(nblk):
                    cb = cfg.chunks_per_block[bb]
                    ohb = ohpool.tile([128, 2 * cbmax * 128], FP8,
                                      tag="ohb", bufs=3)
                    nc.sync.dma_start(
                        out=ohb[:, :2 * cb * 128],
                        in_=ohb_in[:, 2 * off * 128:2 * (off + cb) * 128])
                    ohc = ohb[:, :cb * 128]
                    ohTc = ohb[:, cb * 128:2 * cb * 128]

                    gt = gpool.tile([128, cbmax, ROW], BF16, tag="gt")
                    ca = cfg.ca[bb] if bb < SPLIT_BLKS else 0
                    if 0 < ca < cb:
                        ga = nc.gpsimd.dma_gather(
                            out_ap=gt[:, :ca, :], in_ap=xlrow_t[:],
                            idxs_ap=srcidx_sb[:, off * 8:(off + ca) * 8],
                            num_idxs=128 * ca, num_idxs_reg=nreg(128 * ca),
                            elem_size=ROW, single_packet=128 * ca <= 1024)
                        for w_ in tbl_writes[:40]:
                            add_dep_helper(ga.ins, w_.ins, True, "tblA->g")
                        g_ = nc.gpsimd.dma_gather(
                            out_ap=gt[:, ca:cb, :], in_ap=xlrow_t[:],
                            idxs_ap=srcidx_sb[:, (off + ca) * 8:(off + cb) * 8],
                            num_idxs=128 * (cb - ca),
                            num_idxs_reg=nreg(128 * (cb - ca)),
                            elem_size=ROW,
                            single_packet=128 * (cb - ca) <= 1024)
                        for w_ in tbl_writes:
                            add_dep_helper(g_.ins, w_.ins, True, "tbl->g")
                    else:
                        g_ = nc.gpsimd.dma_gather(
                            out_ap=gt[:, :cb, :], in_ap=xlrow_t[:],
                            idxs_ap=srcidx_sb[:, off * 8:(off + cb) * 8],
                            num_idxs=128 * cb, num_idxs_reg=nreg(128 * cb),
                            elem_size=ROW, single_packet=128 * cb <= 1024)
                        for w_ in tbl_writes:
                            add_dep_helper(g_.ins, w_.ins, True, "tbl->gather")

                    if p1 is not None:
                        stageA(p1)

                    # per-edge alpha_dst via transposed one-hot, batched PSUM
                    ad_den = psS.tile([128, cbmax * heads + heads], FP32,
                                      tag="ad_den")
                    for j in range(cb):
                        nc.tensor.matmul(
                            out=ad_den[:, j * heads:(j + 1) * heads],
                            lhsT=ohTc[:, j * 128:(j + 1) * 128],
                            rhs=ad_own[:, bb * heads:(bb + 1) * heads],
                            start=True, stop=True, skip_group_check=True)
                    sv_all = epool.tile([128, cbmax * heads], BF16, tag="sv")
                    nc.vector.tensor_tensor(
                        out=sv_all[:, :cb * heads].rearrange(
                            "p (a b) -> p a b", a=cb),
                        in0=gt[:, :cb, hc:hc + heads],
                        in1=ad_den[:, :cb * heads].rearrange(
                            "p (a b) -> p a b", a=cb),
                        op=OP.add)
                    # pe = exp(lrelu(sv)); lrelu = max(x, 0.2x) on DVE
                    lr_all = epool.tile([128, cbmax * heads], BF16, tag="lr")
                    nc.vector.tensor_scalar_mul(out=lr_all[:, :cb * heads],
                                                in0=sv_all[:, :cb * heads],
                                                scalar1=NEG_SLOPE)
                    nc.vector.tensor_tensor(out=lr_all[:, :cb * heads],
                                            in0=sv_all[:, :cb * heads],
                                            in1=lr_all[:, :cb * heads],
                                            op=OP.max)
                    pe_all = epool.tile([128, cbmax * heads], BF16, tag="pe")
                    nc.scalar.activation(out=pe_all[:, :cb * heads],
                                         in_=lr_all[:, :cb * heads],
                                         func=AF.Exp)

                    agg = psS.tile([128, hc], FP32, tag="agg")
                    for j in range(cb):
                        first, last = j == 0, j == cb - 1
                        # msg[e, (c,h)] = xl'[e, (c,h)] * pe[e, h]
                        msg = epool.tile([128, hc], BF16, tag="msg", bufs=4)
                        pe_b = (pe_all[:, j * heads:(j + 1) * heads]
                                .rearrange("p (a b) -> p a b", a=1)
                                .to_broadcast([128, HID, heads]))
                        nc.vector.tensor_tensor(
                            out=msg[:].rearrange("p (a b) -> p a b", a=HID),
                            in0=gt[:, j, :hc].rearrange(
                                "p (a b) -> p a b", a=HID),
                            in1=pe_b, op=OP.mult)
                        nc.tensor.matmul(out=ad_den[:, DN:DN + heads],
                                         lhsT=ohc[:, j * 128:(j + 1) * 128],
                                         rhs=pe_all[:, j * heads:(j + 1) * heads],
                                         start=first, stop=last,
                                         skip_group_check=True)
                        for j0 in range(0, hc, 512):
                            j1 = min(j0 + 512, hc)
                            nc.tensor.matmul(out=agg[:, j0:j1],
                                             lhsT=ohc[:, j * 128:(j + 1) * 128],
                                             rhs=msg[:, j0:j1],
                                             start=first, stop=last,
                                             skip_group_check=True)
                    off += cb
                    p1 = {"bb": bb, "agg": agg, "ad_den": ad_den}
                stageA(p1)

                # --- batched layer epilogue over all own nodes ---
                mTall = bpool.tile([128, npc_pad], BF16, tag="mTall",
                                   bufs=1)
                for bb in range(nblk):
                    mT_ps = psS.tile([128, 128], FP32, tag="mT",
                                     name="mT_ps")
                    nc.tensor.transpose(
                        out=mT_ps[:], in_=hmall[:, bb * 128:(bb + 1) * 128],
                        identity=ident_f[:])
                    if bb % 2 == 0:
                        nc.scalar.activation(
                            out=mTall[:, bb * 128:(bb + 1) * 128],
                            in_=mT_ps[:], func=AF.Copy)
                    else:
                        nc.vector.tensor_copy(
                            out=mTall[:, bb * 128:(bb + 1) * 128],
                            in_=mT_ps[:])
                z1a = bpool.tile([128, npc_pad], FP32, tag="z1a", bufs=1)
                nc.scalar.activation(out=z1a[:], in_=mTall[:],
                                     func=AF.Identity,
                                     bias=t_aff[:, li:li + 1],
                                     scale=s_aff[:, li:li + 1])
                z2a = bpool.tile([128, npc_pad], FP32, tag="z2a", bufs=1)
                nc.vector.tensor_scalar_mul(out=z2a[:], in0=hprev[:],
                                            scalar1=ALPHA)
                nc.vector.tensor_tensor(out=z1a[:], in0=z1a[:], in1=z2a[:],
                                        op=OP.add)
                _elu(nc, bpool, hnew[:], z1a[:], (128, npc_pad), "n")

            # ---------------- classifier ----------------
            hfin = h_own[layers % 2]
            zc = bpool.tile([qd, npc_pad], BF16, tag="z2a", bufs=1)
            for j0 in range(0, npc_pad, 512):
                j1 = min(j0 + 512, npc_pad)
                hid_ps = psS.tile([qd, 1024], FP32, tag="agg",
                                  name="hid_ps")
                nc.tensor.matmul(out=hid_ps[:, :j1 - j0], lhsT=W1_sb[:],
                                 rhs=hfin[:, j0:j1], start=True, stop=True)
                nc.scalar.activation(out=zc[:, j0:j1], in_=hid_ps[:, :j1 - j0],
                                     func=AF.Identity,
                                     bias=b1_sb[:, :1], scale=1.0)
            hidsb = bpool.tile([qd, npc_pad], BF16, tag="mTall", bufs=1)
            _elu(nc, bpool, hidsb[:], zc[:], (qd, npc_pad), "n")
            osb = bpool.tile([CLS, npc_pad], FP32, tag="z1a", bufs=1)
            for j0 in range(0, npc_pad, 512):
                j1 = min(j0 + 512, npc_pad)
                out_ps = psS.tile([CLS, 1024], FP32, tag="agg",
                                  name="out_ps")
                nc.tensor.matmul(out=out_ps[:, :j1 - j0], lhsT=W2_sb[:],
                                 rhs=hidsb[:, j0:j1], start=True, stop=True)
                nc.scalar.activation(out=osb[:, j0:j1], in_=out_ps[:, :j1 - j0],
                                     func=AF.Identity,
                                     bias=b2_sb[:, :1], scale=1.0)
            nc.sync.dma_start(out=out_dram[:, :], in_=osb[:])

    return nc


_LAST_EXEC_NS = None


def _run(inputs, trace=False):
    global _LAST_EXEC_NS
    from concourse.bass_utils import run_bass_kernel_spmd

    cfg, shared, per_core = preprocess(**inputs)
    nc = bacc.Bacc("TRN2", target_bir_lowering=False, debug=False,
                   num_devices=M)
    build(nc, cfg)
    nc.compile()

    in_maps = []
    for k in range(M):
        m = dict(shared)
        m.update(per_core[k])
        in_maps.append({k2: np.ascontiguousarray(v) for k2, v in m.items()})

    res = run_bass_kernel_spmd(nc, in_maps, list(range(M)), trace=trace)
    _LAST_EXEC_NS = res.exec_time_ns

    out = np.zeros((N, CLS), np.float32)
    for k in range(M):
        o = res.results[k]["out"]  # [CLS, NPC_PAD]
        out[k * NPC:(k + 1) * NPC] = o[:CLS, :NPC].T
    return out


def kernel(**inputs):
    return _run(inputs, trace=False)
